# revision 11
# baseline (speedup 1.0000x reference)
"""Trainium2 Bass kernel for nn_Autotuner_FFN (dense MLP, 8-core data parallel).

Strategy:
  * Host folds all embedding tables / 57 op-linears / log2-scalings and the
    LayerNorm mean-centerings into one effective first-layer weight matrix
    W1_eff [185, 1024] (+ centered biases). One-hot index encodings become
    extra GEMM rows. Per-element device work shrinks to:
        u = sign(x)*ln(|x|+1) on 57 of 185 feature rows,
        3 GEMMs (185->1024 -> 1024->1024 -> 1024->1),
        2 RMS-style norms (mean already folded) + relu (+g,be affine).
  * Device layout: activations transposed (hidden on partitions, batch on
    free dim). LN stats (sum of squares over hidden) via ones-vector matmul
    on TensorE; rsqrt broadcast back via a rank-1 outer-product matmul.
  * Matmuls run in fp16 (1 cyc/row, fast-weight-load pipelined; fp32 is 4x slower,
    float32r loses ~75ns/MM to serialized 4-byte weight loads).
  * Batch 65536 is sharded 8192/core across 8 NeuronCores (pure DP).
"""
import numpy as np

import concourse.bass as bass
import concourse.tile as tile
from concourse import bacc, mybir
from concourse.bass_utils import run_bass_kernel_spmd

AF = mybir.ActivationFunctionType
ALU = mybir.AluOpType
F32 = mybir.dt.float32
F32R = mybir.dt.float16  # matmul operand dtype: 1 cyc/row + FWL-pipelined weight loads

B = 65536
N_CORES = 8
B_CORE = B // N_CORES          # 8192
CH = 512                       # batch chunk (one PSUM bank wide)
NCH = B_CORE // CH             # 16
HID = 1024
MT = HID // 128                # 8 hidden m-tiles
KA, KC = 128, 57               # feature K tiles (125+3pad | 57 transformed)
EPS = 1e-5
LN2 = float(np.log(2.0))


# ---------------------------------------------------------------- host folds
def _fold_weights(inp):
    f8 = lambda x: np.asarray(x, np.float64)
    W1 = f8(inp["W1"]); b1 = f8(inp["b1"])
    emb_kc = f8(inp["emb_kc"]); emb_nl = f8(inp["emb_nl"])
    op_W = f8(inp["op_W"]); op_b = f8(inp["op_b"])
    emb_c = f8(inp["emb_contig"]); emb_s = f8(inp["emb_scalar"])
    emb_i = f8(inp["emb_indirect"])
    H = W1.shape[1]
    rows_A = []
    bias = b1.copy()
    rows_A.append(emb_kc @ W1[0:16])
    rows_A.append(emb_nl @ W1[16:32])
    W1_op = W1[32:944].reshape(57, 16, H)
    rows_A.append(np.einsum("ij,ijh->ih", op_W, W1_op))
    bias += np.einsum("ij,ijh->h", op_b, W1_op)
    rd_f2, rd_bool, rd_ss = [], [], []
    wd_f2, wd_bool, wd_ss = [], [], []
    for base, f2l, booll, ssl in ((947, rd_f2, rd_bool, rd_ss),
                                  (1027, wd_f2, wd_bool, wd_ss)):
        for d in range(4):
            Wd = W1[base + 20 * d: base + 20 * d + 20]
            f2l.append(Wd[0:2])
            ssl.append(Wd[2:8] / LN2)
            rows_b = []
            for e, sl in ((emb_c, slice(8, 12)), (emb_s, slice(12, 16)),
                          (emb_i, slice(16, 20))):
                rows_b.append((e[1] - e[0]) @ Wd[sl])
                bias += e[0] @ Wd[sl]
            booll.append(np.stack(rows_b))
    rows_A += [np.concatenate(rd_f2), np.concatenate(rd_bool),
               np.concatenate(wd_f2), np.concatenate(wd_bool),
               W1[1110:1112]]
    A = np.concatenate(rows_A)
    C = np.concatenate([W1[944:947] / LN2, W1[1107:1110] / LN2,
                        W1[1112:1115] / LN2,
                        np.concatenate(rd_ss), np.concatenate(wd_ss)])
    W1_eff = np.concatenate([A, np.zeros((3, H)), C])       # [185, H]
    W1c = W1_eff - W1_eff.mean(axis=1, keepdims=True)
    bc1 = bias - bias.mean()
    W2 = f8(inp["W2"]); b2 = f8(inp["b2"])
    W2c = W2 - W2.mean(axis=1, keepdims=True)
    bc2 = b2 - b2.mean()
    return (W1c.astype(np.float32), bc1.astype(np.float32),
            W2c.astype(np.float32), bc2.astype(np.float32))


def _build_xt(inp):
    Bn = inp["op_vec"].shape[0]
    kc = np.asarray(inp["kernel_category_idx"]).astype(np.int64)
    nl = np.asarray(inp["num_of_loops_idx"]).astype(np.int64)
    f = lambda k: np.asarray(inp[k], np.float32)
    XT = np.zeros((KA + KC, Bn), np.float32)
    XT[0:10] = (np.arange(10)[:, None] == kc[None, :])
    XT[10:26] = (np.arange(16)[:, None] == nl[None, :])
    XT[26:83] = f("op_vec").T
    XT[83:91] = f("read_dep_float")[:, :, 0:2].reshape(Bn, 8).T
    XT[91:103] = np.asarray(inp["read_dep_bools"]).reshape(Bn, 12).T
    XT[103:111] = f("write_dep_float")[:, :, 0:2].reshape(Bn, 8).T
    XT[111:123] = np.asarray(inp["write_dep_bools"]).reshape(Bn, 12).T
    XT[123:125] = f("rest_vec")[:, 3:5].T
    XT[128:131] = f("size_hints").T
    XT[131:137] = f("rest_vec")[:, [0, 1, 2, 5, 6, 7]].T
    XT[137:161] = f("read_dep_float")[:, :, 2:8].reshape(Bn, 24).T
    XT[161:185] = f("write_dep_float")[:, :, 2:8].reshape(Bn, 24).T
    return XT


def _pack128(v):
    """[1024] -> [128, 8] with v[m*128+p] at [p, m]."""
    return np.ascontiguousarray(np.asarray(v, np.float32).reshape(8, 128).T)


# ---------------------------------------------------------------- device prog
DEFAULT_CFG = dict(h_bufs=1, sq_bufs=1, r1_bufs=1, r2_bufs=1,
                   ps_mm_bufs=3, xin_bufs=3, xr_bufs=2, per_m=False, l2_fp16=False)


def build_program(simple_affine, loop_iters=None, cfg=None):
    """Build the per-core bass program. simple_affine: g==1 and be==0."""
    cfg = {**DEFAULT_CFG, **(cfg or {})}
    nc = bacc.Bacc("TRN2", target_bir_lowering=False, debug=False)
    xt = nc.dram_tensor("xt", [KA + KC, B_CORE], F32, kind="ExternalInput")
    w1 = nc.dram_tensor("w1", [KA + KC, HID], F32, kind="ExternalInput")
    w2 = nc.dram_tensor("w2", [HID, HID], F32, kind="ExternalInput")
    w3p = nc.dram_tensor("w3p", [128, MT], F32, kind="ExternalInput")
    bc1p = nc.dram_tensor("bc1p", [128, MT], F32, kind="ExternalInput")
    bc2p = nc.dram_tensor("bc2p", [128, MT], F32, kind="ExternalInput")
    g1p = nc.dram_tensor("g1p", [128, MT], F32, kind="ExternalInput")
    be1p = nc.dram_tensor("be1p", [128, MT], F32, kind="ExternalInput")
    g2p = nc.dram_tensor("g2p", [128, MT], F32, kind="ExternalInput")
    be2p = nc.dram_tensor("be2p", [128, MT], F32, kind="ExternalInput")
    b3t = nc.dram_tensor("b3t", [1, 1], F32, kind="ExternalInput")
    y = nc.dram_tensor("y", [1, B_CORE], F32, kind="ExternalOutput")

    from contextlib import ExitStack
    with tile.TileContext(nc) as tc, ExitStack() as ctx, \
            nc.allow_low_precision(reason="f32r rounding is intentional"):
        const = ctx.enter_context(tc.tile_pool(name="const", bufs=1))
        wstage = ctx.enter_context(tc.tile_pool(name="wstage", bufs=2))
        xin = ctx.enter_context(tc.tile_pool(name="xin", bufs=cfg["xin_bufs"]))
        xr = ctx.enter_context(tc.tile_pool(name="xr", bufs=cfg["xr_bufs"]))
        bigH = ctx.enter_context(tc.tile_pool(name="bigH", bufs=cfg["h_bufs"]))
        bigS = ctx.enter_context(tc.tile_pool(name="bigS", bufs=cfg["sq_bufs"]))
        bigR1 = ctx.enter_context(tc.tile_pool(name="bigR1", bufs=cfg["r1_bufs"]))
        bigR2 = ctx.enter_context(tc.tile_pool(name="bigR2", bufs=cfg["r2_bufs"]))
        small = ctx.enter_context(tc.tile_pool(name="small", bufs=2))
        ps_mm = ctx.enter_context(tc.tile_pool(name="ps_mm", bufs=cfg["ps_mm_bufs"], space="PSUM"))
        ps_st = ctx.enter_context(tc.tile_pool(name="ps_st", bufs=2, space="PSUM"))
        ps_vec = ctx.enter_context(tc.tile_pool(name="ps_vec", bufs=2, space="PSUM"))

        # ---- one-time setup: load + round weights to f32r
        w1a_r = const.tile([128, HID], F32R, tag="w1a")
        st = wstage.tile([128, HID], F32, tag="stage")
        nc.sync.dma_start(st[:], w1.ap()[0:128, :])
        nc.vector.tensor_copy(w1a_r[:], st[:])
        w1c_r = const.tile([KC, HID], F32R, tag="w1c")
        stc = wstage.tile([KC, HID], F32, tag="stagec")
        nc.sync.dma_start(stc[:], w1.ap()[128:185, :])
        nc.vector.tensor_copy(w1c_r[:], stc[:])
        L2DT = mybir.dt.float16 if cfg["l2_fp16"] else F32R
        w2r = []
        for k in range(MT):
            stk = wstage.tile([128, HID], F32, tag="stage")
            nc.sync.dma_start(stk[:], w2.ap()[k * 128:(k + 1) * 128, :])
            t = const.tile([128, HID], L2DT, tag=f"w2r{k}")
            nc.vector.tensor_copy(t[:], stk[:])
            w2r.append(t)
        w3p_r = const.tile([128, MT], L2DT, tag="w3p")
        st3 = wstage.tile([128, MT], F32, tag="stages")
        nc.sync.dma_start(st3[:], w3p.ap())
        nc.vector.tensor_copy(w3p_r[:], st3[:])

        def load_small(name, dram):
            t = const.tile([128, MT], F32, tag=name)
            nc.sync.dma_start(t[:], dram.ap())
            return t
        bc1s = load_small("bc1s", bc1p); bc2s = load_small("bc2s", bc2p)
        g1s = load_small("g1s", g1p); be1s = load_small("be1s", be1p)
        g2s = load_small("g2s", g2p); be2s = load_small("be2s", be2p)
        b3s = const.tile([1, 1], F32, tag="b3s")
        nc.sync.dma_start(b3s[:], b3t.ap())
        ones_st = const.tile([128, 1], F32, tag="ones_st")
        nc.vector.memset(ones_st[:], 1.0)
        ones_col = const.tile([128, 1], F32R, tag="ones_col")
        nc.vector.tensor_copy(ones_col[:], ones_st[:])
        ones_rst = const.tile([1, 128], F32, tag="ones_rst")
        nc.vector.memset(ones_rst[:], 1.0)
        ones_row = const.tile([1, 128], F32R, tag="ones_row")
        nc.vector.tensor_copy(ones_row[:], ones_rst[:])
        eps_t = const.tile([1, 1], F32, tag="eps_t")
        nc.vector.memset(eps_t[:], EPS)

        def layer_norm_relu(Hb, g_s, be_s, out_pool, out_tag):
            """Hb [128, MT*CH] f32 (centered pre-act). Returns relu'd f32r."""
            sqb = bigS.tile([128, MT * CH], F32R, tag="sq")
            if cfg["per_m"]:
                for m in range(MT):
                    sl = slice(m * CH, (m + 1) * CH)
                    nc.vector.tensor_mul(sqb[:, sl], Hb[:, sl], Hb[:, sl])
            else:
                nc.vector.tensor_mul(sqb[:], Hb[:], Hb[:])
            pst = ps_st.tile([1, CH], F32, tag="pst")
            for m in range(MT):
                nc.tensor.matmul(pst[:], ones_col[:],
                                 sqb[:, m * CH:(m + 1) * CH],
                                 start=(m == 0), stop=(m == MT - 1))
            sd = small.tile([1, CH], F32, tag="sd")
            nc.scalar.activation(sd[:], pst[:], AF.Sqrt,
                                 bias=eps_t[:], scale=1.0 / HID)
            rs = small.tile([1, CH], F32R, tag="rs")
            nc.vector.reciprocal(rs[:], sd[:])
            pv = ps_vec.tile([128, CH], F32, tag="pv")
            nc.tensor.matmul(pv[:], ones_row[:], rs[:], start=True, stop=True)
            Rb = out_pool.tile([128, MT * CH], L2DT, tag=out_tag)
            if cfg["per_m"]:
                for m in range(MT):
                    sl = slice(m * CH, (m + 1) * CH)
                    nc.vector.tensor_mul(Hb[:, sl], Hb[:, sl], pv[:])
                    if simple_affine:
                        nc.scalar.activation(Rb[:, sl], Hb[:, sl], AF.Relu)
                    else:
                        nc.scalar.activation(Rb[:, sl], Hb[:, sl], AF.Relu,
                                             bias=be_s[:, m:m + 1],
                                             scale=g_s[:, m:m + 1])
            else:
                h3 = Hb[:].rearrange("p (m n) -> p m n", m=MT)
                pvb = bass.AP(tensor=pv[:].tensor, offset=pv[:].offset,
                              ap=[pv[:].ap[0], [0, MT], pv[:].ap[1]])
                nc.vector.tensor_mul(h3, h3, pvb)
                if simple_affine:
                    nc.scalar.activation(Rb[:], Hb[:], AF.Relu)
                else:
                    for m in range(MT):
                        sl = slice(m * CH, (m + 1) * CH)
                        nc.scalar.activation(Rb[:, sl], Hb[:, sl], AF.Relu,
                                             bias=be_s[:, m:m + 1],
                                             scale=g_s[:, m:m + 1])
            return Rb

        def chunk_body(c):
            x1 = xin.tile([128, CH], F32, tag="x1")
            nc.sync.dma_start(x1[:], xt.ap()[0:128, c * CH:(c + 1) * CH])
            x2 = xin.tile([KC, CH], F32, tag="x2")
            nc.sync.dma_start(x2[:], xt.ap()[128:185, c * CH:(c + 1) * CH])
            x1r = xr.tile([128, CH], F32R, tag="x1r")
            nc.vector.tensor_copy(x1r[:], x1[:])
            # u = sign(x)*ln(|x|+1) on the 57 transformed rows
            xab = xr.tile([KC, CH], F32, tag="xab")
            nc.vector.tensor_scalar(
                out=xab[:].bitcast(mybir.dt.int32),
                in0=x2[:].bitcast(mybir.dt.int32),
                scalar1=0x7FFFFFFF, scalar2=None, op0=ALU.bitwise_and)
            xln = xr.tile([KC, CH], F32, tag="xln")
            nc.scalar.activation(xln[:], xab[:], AF.Ln, bias=1.0)
            xsg = xr.tile([KC, CH], F32, tag="xsg")
            nc.scalar.activation(xsg[:], x2[:], AF.Sign)
            x2r = xr.tile([KC, CH], F32R, tag="x2r")
            nc.vector.tensor_mul(x2r[:], xsg[:], xln[:])

            # ---- layer 1
            H1 = bigH.tile([128, MT * CH], F32, tag="H")
            for m in range(MT):
                p1 = ps_mm.tile([128, CH], F32, tag="pmm")
                nc.tensor.matmul(p1[:], w1a_r[:, m * 128:(m + 1) * 128],
                                 x1r[:], start=True, stop=False)
                nc.tensor.matmul(p1[:], w1c_r[:, m * 128:(m + 1) * 128],
                                 x2r[:], start=False, stop=True)
                nc.scalar.activation(H1[:, m * CH:(m + 1) * CH], p1[:],
                                     AF.Identity, bias=bc1s[:, m:m + 1])
            R1 = layer_norm_relu(H1, g1s, be1s, bigR1, "R1")

            # ---- layer 2
            H2 = bigH.tile([128, MT * CH], F32, tag="H")
            for m in range(MT):
                p2 = ps_mm.tile([128, CH], F32, tag="pmm")
                for k in range(MT):
                    nc.tensor.matmul(p2[:], w2r[k][:, m * 128:(m + 1) * 128],
                                     R1[:, k * CH:(k + 1) * CH],
                                     start=(k == 0), stop=(k == MT - 1))
                nc.scalar.activation(H2[:, m * CH:(m + 1) * CH], p2[:],
                                     AF.Identity, bias=bc2s[:, m:m + 1])
            R2 = layer_norm_relu(H2, g2s, be2s, bigR2, "R2")

            # ---- layer 3
            p3 = ps_st.tile([1, CH], F32, tag="pst")
            for k in range(MT):
                nc.tensor.matmul(p3[:], w3p_r[:, k:k + 1],
                                 R2[:, k * CH:(k + 1) * CH],
                                 start=(k == 0), stop=(k == MT - 1))
            osb = small.tile([1, CH], F32, tag="osb")
            nc.scalar.activation(osb[:], p3[:], AF.Identity, bias=b3s[:])
            nc.sync.dma_start(y.ap()[0:1, c * CH:(c + 1) * CH], osb[:])

        if loop_iters is None:
            for c in range(NCH):
                chunk_body(c)
        else:
            with tc.For_i(0, loop_iters, 1):
                for c in range(NCH):
                    chunk_body(c)
    nc.compile()
    return nc


# ---------------------------------------------------------------- entry point
_CACHE = {}


BEST_CFG = dict(per_m=True, h_bufs=2, ps_mm_bufs=4)


def _get_program(simple_affine):
    key = ("prog", simple_affine)
    if key not in _CACHE:
        _CACHE[key] = build_program(simple_affine, cfg=BEST_CFG)
    return _CACHE[key]


def make_in_maps(inputs):
    inp = {k: np.asarray(v) for k, v in inputs.items()}
    W1c, bc1, W2c, bc2 = _fold_weights(inp)
    XT = _build_xt(inp)
    g1 = np.asarray(inp["g1"], np.float32); be1 = np.asarray(inp["be1"], np.float32)
    g2 = np.asarray(inp["g2"], np.float32); be2 = np.asarray(inp["be2"], np.float32)
    simple_affine = bool(
        np.all(g1 == 1.0) and np.all(g2 == 1.0)
        and np.all(be1 == 0.0) and np.all(be2 == 0.0))
    W3 = np.asarray(inp["W3"], np.float32)
    b3 = np.asarray(inp["b3"], np.float32)
    shared = {
        "w1": W1c, "w2": W2c,
        "w3p": _pack128(W3[:, 0]),
        "bc1p": _pack128(bc1), "bc2p": _pack128(bc2),
        "g1p": _pack128(g1), "be1p": _pack128(be1),
        "g2p": _pack128(g2), "be2p": _pack128(be2),
        "b3t": b3.reshape(1, 1),
    }
    in_maps = []
    for c in range(N_CORES):
        m = dict(shared)
        m["xt"] = np.ascontiguousarray(XT[:, c * B_CORE:(c + 1) * B_CORE])
        in_maps.append(m)
    return in_maps, simple_affine


def kernel(**inputs) -> np.ndarray:
    in_maps, simple_affine = make_in_maps(inputs)
    nc = _get_program(simple_affine)
    res = run_bass_kernel_spmd(nc, in_maps, core_ids=list(range(N_CORES)))
    y = np.concatenate([r["y"][0] for r in res.results])
    return y.reshape(B, 1).astype(np.float32)


if __name__ == "__main__":
    import jax
    import reference
    cpu = jax.devices("cpu")[0]
    with jax.default_device(cpu):
        inp = reference.setup_inputs()
        ref = np.asarray(reference.reference(**inp))
    out = kernel(**{k: np.asarray(v) for k, v in inp.items()})
    err = np.abs(out - ref)
    scale = np.abs(ref).max()
    print("max_abs", err.max(), "rel(vs scale)", err.max() / scale,
          "mean_rel", (err / (np.abs(ref) + 1e-6)).mean())
